# revision 9
# baseline (speedup 1.0000x reference)
"""Trainium2 Bass kernel for BatchRankingLoss.

Reference (B=131072, d=256 decoys, K=512 complexes, G=K-1=511 groups):
    o, t -> [G, d]; dt = t_i - t_j; y = sign(dt); w = |dt| > 0.1
    dL = w * max(0, 1 + y*(o_i - o_j)); loss = sum(dL) / (G*d*(d-1))

Symmetry: dL(i,j) == dL(j,i) for active pairs, so
    sum(dL) = 2 * sum_{t_i - t_j > 0.1} relu(1 + o_i - o_j)

Device layout (groups host-sorted by t within each group, so active pairs
have j < i): 64 groups/core, partition p = (g_local, par); slice s covers
i in {4s..4s+3} via par plus 2 column-interleaved i-slots e; free col =
2*j + e, extent 2*J_s.

Per slice:
  PE:   D = (1 + o_i) - o_j  over j in [0, J_s)        -> PSUM chunk
        U = (t_i - 0.1) - t_j over j in [L_s, J_s)     -> packed band PSUM
  relu: rD = relu(D) (chunk-merged, accum_out -> S1)   on ACT/DVE/Pool
  stt:  S2[s] = sum (U <= 0) * rD over the band        on DVE/Pool
Host: loss = 2 * (sum S1 - sum S2) / N.

Extents are exact for any input: with tbf = bf16(t), tbv = bf16(t - 0.1),
c[g,i] = #{j: tbf[j] < tbv[i]} bounds all active pairs; J_s >= max c,
L_s <= min c (margin 2). The padded group 512 is neutralized by setting
its value rows to 1 + o := -1000 (relu always 0) and t rows to 0.
"""

import numpy as np
from contextlib import ExitStack

import concourse.bacc as bacc
import concourse.mybir as mybir
import concourse.tile as tile
from concourse.bass_utils import run_bass_kernel_spmd
import ml_dtypes

BF16 = ml_dtypes.bfloat16

N_CORES = 8
D = 256
G_REAL = 511
G_PAD = 512
GPC = 64                  # groups per core
NS = 64                   # slices per core (4 i-values each)
KDIM = 68                 # 64 indicator rows + 2 t-value rows + 2 o-value rows
N_PAIRS = G_REAL * D * (D - 1)
MARGIN = 2
PSUM_F32 = 512            # f32 columns per PSUM bank

_CACHED = {}


def _pack_chunks(exts, cap=PSUM_F32):
    """Greedy-pack slice indices into chunks with total extent <= cap."""
    chunks, cur, tot = [], [], 0
    for s, e in enumerate(exts):
        if e == 0:
            continue
        if tot + e > cap and cur:
            chunks.append(cur)
            cur, tot = [], 0
        cur.append(s)
        tot += e
    if cur:
        chunks.append(cur)
    return chunks


def _assign_engines(chunks, band_chunks, ES, EBS):
    """Static load balance. Returns (relu_eng per chunk, stt_eng per slice).

    Cost model (ns): per-column rate + per-instruction fixed cost.
      ACT 0.833/col + 150   (relu only)
      DVE 1.042/col + 90
      Pool 1.39/col + 140
    """
    load = {"act": 0.0, "dve": 0.0, "pool": 0.0}
    rate = {"act": 0.833, "dve": 1.042, "pool": 1.39}
    fixed = {"act": 150.0, "dve": 90.0, "pool": 140.0}

    relu_eng = {}
    stt_eng = {}
    # Interleave assignment in execution order: chunk relu, then its stts.
    sl2bc = {}
    for bi, bc in enumerate(band_chunks):
        for s in bc:
            sl2bc[s] = bi
    # GPSIMD/Pool cannot access PSUM, so it gets neither op here.
    for ci, ch in enumerate(chunks):
        cols = sum(ES[s] for s in ch)
        best = min(("act", "dve"),
                   key=lambda e: load[e] + cols * rate[e] + fixed[e])
        relu_eng[ci] = best
        load[best] += cols * rate[best] + fixed[best]
        for s in ch:
            if EBS[s] == 0:
                continue
            best = "dve"
            stt_eng[s] = best
            load[best] += EBS[s] * rate[best] + fixed[best]
    return relu_eng, stt_eng, load


def _build_program(JS, LS, repeat=1, mode="band"):
    """JS/LS: per-slice j extents (ints). mode: "band" or "full"
    (full: U over the whole extent, stt is_gt everywhere, no S1)."""
    nc = bacc.Bacc("TRN2", target_bir_lowering=False, debug=False,
                   num_devices=N_CORES)
    f32 = mybir.dt.float32
    bf16 = mybir.dt.bfloat16
    A = mybir.AluOpType

    ES = [2 * j for j in JS]                       # relu extents
    if mode == "full":
        LS = [0] * NS
    EBS = [2 * (JS[s] - LS[s]) if JS[s] > 0 else 0 for s in range(NS)]

    chunks = _pack_chunks(ES)
    band_chunks = _pack_chunks(EBS)
    relu_eng, stt_eng, _ = _assign_engines(chunks, band_chunks, ES, EBS)
    NCH = len(chunks)
    assert 64 + NCH <= 128

    ind_d = nc.dram_tensor("ind", [GPC, 128], bf16, kind="ExternalInput")
    wvt_d = nc.dram_tensor("wv_t", [2, NS * 128], bf16, kind="ExternalInput")
    wvo_d = nc.dram_tensor("wv_o", [2, NS * 128], bf16, kind="ExternalInput")
    rt_d = nc.dram_tensor("rt", [KDIM, 512], bf16, kind="ExternalInput")
    ro_d = nc.dram_tensor("ro", [KDIM, 512], bf16, kind="ExternalInput")
    macc_d = nc.dram_tensor("m_acc", [128, 128], f32, kind="ExternalOutput")

    with ExitStack() as ctx:
        tc = ctx.enter_context(tile.TileContext(nc, num_cores=N_CORES))
        consts = ctx.enter_context(tc.tile_pool(name="consts", bufs=1))
        psd_pool = ctx.enter_context(tc.tile_pool(name="psd", bufs=3, space="PSUM"))
        psu_pool = ctx.enter_context(tc.tile_pool(name="psu", bufs=3, space="PSUM"))
        rd_pool = ctx.enter_context(tc.tile_pool(name="rd", bufs=4))
        junk_pool = ctx.enter_context(tc.tile_pool(name="junk", bufs=2))

        w = consts.tile([KDIM, NS * 128], bf16)
        g_small = consts.tile([GPC, 128], bf16)
        rt = consts.tile([KDIM, 512], bf16)
        ro = consts.tile([KDIM, 512], bf16)
        macc = consts.tile([128, 128], f32)

        nc.sync.dma_start(g_small[:], ind_d[:])
        nc.sync.dma_start(rt[:], rt_d[:])
        nc.sync.dma_start(ro[:], ro_d[:])
        nc.sync.dma_start(w[GPC:GPC + 2, :], wvt_d[:])
        nc.sync.dma_start(w[GPC + 2:KDIM, :], wvo_d[:])
        nc.vector.memset(macc[:], 0.0)

        # Replicate the indicator block across the 64 slice stationaries,
        # split over engines so early slices unblock fast.
        PIECES = [(nc.vector, 0, 16), (nc.gpsimd, 16, 32),
                  (nc.scalar, 32, 48), (nc.vector, 48, 64)]
        for engine, s0, s1 in PIECES:
            n = s1 - s0
            src = g_small[:, None, :].broadcast_to((GPC, n, 128))
            dst = w[0:GPC, s0 * 128:s1 * 128]
            if engine is nc.scalar:
                engine.copy(dst.rearrange("g (r p) -> g r p", p=128), src)
            else:
                engine.tensor_copy(dst.rearrange("g (r p) -> g r p", p=128), src)

        ENG = {"act": nc.scalar, "dve": nc.vector, "pool": nc.gpsimd}

        for _rep in range(repeat):
            # map slice -> (band chunk idx, offset)
            band_off = {}
            for bi, bc in enumerate(band_chunks):
                off = 0
                for s in bc:
                    band_off[s] = (bi, off)
                    off += EBS[s]

            bc_tiles = {}
            for ci, ch in enumerate(chunks):
                ext = sum(ES[s] for s in ch)
                psd = psd_pool.tile([128, PSUM_F32], f32, tag="d")
                off = 0
                offs = {}
                for s in ch:
                    offs[s] = off
                    nc.tensor.matmul(
                        psd[:, off:off + ES[s]],
                        lhsT=w[0:KDIM, s * 128:(s + 1) * 128],
                        rhs=ro[0:KDIM, 0:ES[s]],
                        start=True, stop=True,
                    )
                    off += ES[s]

                # band U matmuls for this chunk's slices (band chunks span
                # several D chunks; allocate their PSUM tile on first use)
                for s in ch:
                    if EBS[s] == 0:
                        continue
                    bi, boff = band_off[s]
                    if bi not in bc_tiles:
                        bc_tiles[bi] = psu_pool.tile(
                            [128, PSUM_F32], f32, tag="u", name=f"psu{bi}")
                    nc.tensor.matmul(
                        bc_tiles[bi][:, boff:boff + EBS[s]],
                        lhsT=w[0:KDIM, s * 128:(s + 1) * 128],
                        rhs=rt[0:KDIM, 2 * LS[s]:2 * JS[s]],
                        start=True, stop=True,
                    )

                rd = rd_pool.tile([128, PSUM_F32], bf16, tag="rd")
                e = relu_eng[ci]
                if mode == "band":
                    acc = macc[:, 64 + ci:64 + ci + 1]
                else:
                    acc = None
                if e == "act":
                    nc.scalar.activation(
                        rd[:, 0:ext], psd[:, 0:ext],
                        mybir.ActivationFunctionType.Relu, accum_out=acc)
                else:
                    ENG[e].tensor_scalar(
                        out=rd[:, 0:ext], in0=psd[:, 0:ext],
                        scalar1=0.0, scalar2=0.0, op0=A.max, op1=A.add,
                        accum_out=acc)

                # stt corrections for slices of this chunk
                for s in ch:
                    if EBS[s] == 0:
                        continue
                    bi, boff = band_off[s]
                    psu = bc_tiles[bi]
                    junk = junk_pool.tile([128, PSUM_F32], bf16, tag="j")
                    e2 = stt_eng[s]
                    op0 = A.is_le if mode == "band" else A.is_gt
                    ENG[e2].scalar_tensor_tensor(
                        out=junk[:, 0:EBS[s]],
                        in0=psu[:, boff:boff + EBS[s]],
                        scalar=0.0,
                        in1=rd[:, offs[s] + 2 * LS[s]:offs[s] + 2 * JS[s]],
                        op0=op0, op1=A.mult,
                        accum_out=macc[:, s:s + 1],
                    )

        nc.sync.dma_start(macc_d[:], macc[:])

    nc.compile()
    return nc, NCH


def _host_prep(t_all, o_all):
    """Sort each group by t; return per-core slabs + global extents."""
    t_g = np.zeros((G_PAD, D), dtype=np.float32)
    o_g = np.zeros((G_PAD, D), dtype=np.float32)
    t_g[:G_REAL] = t_all.reshape(G_REAL, D)
    o_g[:G_REAL] = o_all.reshape(G_REAL, D)
    idx = np.argsort(t_g, axis=1)
    t_g = np.take_along_axis(t_g, idx, axis=1)
    o_g = np.take_along_axis(o_g, idx, axis=1)

    # exact active-pair bounds in bf16 semantics
    tbf = t_g.astype(BF16).astype(np.float32)
    tbv = (t_g - np.float32(0.1)).astype(BF16).astype(np.float32)
    c = np.empty((G_PAD, D), dtype=np.int64)
    for g in range(G_REAL):
        c[g] = np.searchsorted(tbf[g], tbv[g], side="left")
    c[G_REAL:] = 0

    cr = c[:G_REAL].reshape(G_REAL, NS, 4)
    cmax = cr.max(axis=(0, 2))          # [NS]
    cmin = cr.min(axis=(0, 2))          # [NS]
    JS, LS = [], []
    for s in range(NS):
        j_static = min(D, 4 * s + 4)
        j = min(j_static, int(cmax[s]) + MARGIN)
        if int(cmax[s]) == 0:
            j = 0                        # no active pairs in this slice
        l = max(0, min(int(cmin[s]) - MARGIN, j))
        JS.append(j)
        LS.append(l)
    return t_g, o_g, JS, LS


def _prep_core_inputs(t_c, o_c, is_last_core):
    """t_c/o_c: [GPC, D] f32 sorted slabs for this core."""
    tv = (t_c - np.float32(0.1)).astype(BF16)
    ov = (np.float32(1.0) + o_c).astype(BF16)
    if is_last_core:
        ov[GPC - 1, :] = BF16(-1000.0)   # neutralize padded group

    # value rows: [e, s*128 + (g*2+par)] = val[g, 4s+2e+par]
    def vrows(v):
        a = v.reshape(GPC, NS, 2, 2)          # [g, s, e, par]
        a = a.transpose(2, 1, 0, 3)           # [e, s, g, par]
        return np.ascontiguousarray(a.reshape(2, NS * 128))

    ind = (np.arange(128)[None, :] // 2 ==
           np.arange(GPC)[:, None]).astype(BF16)

    rt = np.zeros((KDIM, 512), dtype=BF16)
    ro = np.zeros((KDIM, 512), dtype=BF16)
    mt = (-t_c).astype(BF16)
    mo = (-o_c).astype(BF16)
    rt[:GPC, 0::2] = mt
    rt[:GPC, 1::2] = mt
    ro[:GPC, 0::2] = mo
    ro[:GPC, 1::2] = mo
    rt[GPC, 0::2] = BF16(1.0)
    rt[GPC + 1, 1::2] = BF16(1.0)
    ro[GPC + 2, 0::2] = BF16(1.0)
    ro[GPC + 3, 1::2] = BF16(1.0)

    return {"ind": ind, "wv_t": vrows(tv), "wv_o": vrows(ov),
            "rt": rt, "ro": ro}


def combine(res, mode="band"):
    total = np.float64(0.0)
    for c in range(N_CORES):
        m = res.results[c]["m_acc"].astype(np.float64)
        if mode == "band":
            total += m[:, 64:].sum() - m[:, :64].sum()
        else:
            total += m[:, :64].sum()
    return 2.0 * total / float(N_PAIRS)


def kernel(input, gdt_ts):
    o_all = np.asarray(input).reshape(-1)[: G_REAL * D].astype(np.float32, copy=False)
    t_all = np.asarray(gdt_ts).reshape(-1)[: G_REAL * D].astype(np.float32, copy=False)

    t_g, o_g, JS, LS = _host_prep(t_all, o_all)

    key = ("band", tuple(JS), tuple(LS))
    if _CACHED.get("key") != key:
        nc, nch = _build_program(JS, LS, mode="band")
        _CACHED.update(key=key, nc=nc, nch=nch)
    nc = _CACHED["nc"]

    in_maps = []
    for c in range(N_CORES):
        sl = slice(c * GPC, (c + 1) * GPC)
        in_maps.append(_prep_core_inputs(t_g[sl], o_g[sl], c == N_CORES - 1))

    res = run_bass_kernel_spmd(nc, in_maps, list(range(N_CORES)))
    loss = combine(res, mode="band")
    return np.array([loss], dtype=np.float32)


# revision 18
# speedup vs baseline: 528.3912x; 528.3912x over previous
"""BatchRankingLoss on TRN2 — vector-engine-only design (no PE/PSUM).

loss = 2/N * sum_{t_i - t_j > 0.1} relu(1 + o_i - o_j)   (pair symmetry)

Per core: 64 groups sorted by t; partition p = (g, par), slice s <-> i = 2s+par.
Active pairs have j < i (sorted), exactly j < c[g,i] with
c = #{j: bf16(t_j) < bf16(t_i - 0.1)} (computed on host from t only).

Device work:
  1. per slice s: rd_all[:, off_s:off_s+J_s] = relu(BC[:,s] + OJN[:, :J_s])
     -- DVE tensor_scalar (bf16 SBUF, 4x mode), per-partition scalar AP.
     J_s = max_p c + 1 covers all active pairs.
  2. accum pass: big tensor_scalar/activation chunks over rd_all with
     accum_out -> S1 (includes inactive cols c_p <= j < J_s).
  3. correction: host packs the inactive band cells as bf16(b - o_j)
     (active cells -> -100) into OB; device relu+accum -> S2.
loss = 2 * (S1 - S2) / N. Bf16 rounding differences between S1's and S2's
band terms are random +-0.4% on ~3% of pairs — far inside the 2e-2 gate.
"""

import os
import numpy as np
from contextlib import ExitStack

import concourse.bacc as bacc
import concourse.mybir as mybir
import concourse.tile as tile
from concourse.bass_utils import run_bass_kernel_spmd
import ml_dtypes

BF16 = ml_dtypes.bfloat16

N_CORES = 8
D = 256
G_REAL = 511
G_PAD = 512
GPC = 64
NS = 128                  # slices per core; slice s <-> i = 2s + par
N_PAIRS = G_REAL * D * (D - 1)
MARGIN = 2
ACHUNK = 2048             # accum-pass chunk columns
_CACHED = {}


def _build_program(JS, LS, BW, repeat=1, loop=0, accum_eng="dve"):
    """JS/LS: per-slice j extents. BW: total band columns (packed OB)."""
    nc = bacc.Bacc("TRN2", target_bir_lowering=False, debug=False,
                   num_devices=N_CORES)
    f32 = mybir.dt.float32
    bf16 = mybir.dt.bfloat16
    A = mybir.AluOpType

    offs = np.concatenate([[0], np.cumsum(JS)]).astype(int)
    RTOT = int(offs[-1])              # rd_all columns
    live = [s for s in range(NS) if JS[s] > 0]

    # accumulation chunks over rd_all
    achunks = []
    a0 = 0
    while a0 < RTOT:
        achunks.append((a0, min(RTOT, a0 + ACHUNK)))
        a0 += ACHUNK
    bchunks = []
    b0 = 0
    while b0 < BW:
        bchunks.append((b0, min(BW, b0 + ACHUNK)))
        b0 += ACHUNK
    NACC = len(achunks) + len(bchunks)
    assert NACC <= 64

    ojn_d = nc.dram_tensor("ojn", [128, D], bf16, kind="ExternalInput")
    bc_d = nc.dram_tensor("bc", [128, NS], f32, kind="ExternalInput")
    ob_d = nc.dram_tensor("ob", [128, max(BW, 1)], bf16, kind="ExternalInput")
    macc_d = nc.dram_tensor("m_acc", [128, 64], f32, kind="ExternalOutput")

    with ExitStack() as ctx:
        tc = ctx.enter_context(tile.TileContext(nc, num_cores=N_CORES))
        consts = ctx.enter_context(tc.tile_pool(name="consts", bufs=1))
        rd_pool = ctx.enter_context(tc.tile_pool(name="rdp", bufs=2))
        scrap_pool = ctx.enter_context(tc.tile_pool(name="scrap", bufs=3))

        ojn = consts.tile([128, D], bf16)
        bc = consts.tile([128, NS], f32)
        ob = consts.tile([128, max(BW, 1)], bf16)
        macc = consts.tile([128, 64], f32)

        nc.sync.dma_start(ojn[:], ojn_d[:])
        nc.sync.dma_start(bc[:], bc_d[:])
        if BW > 0:
            half = (BW // 2) & ~3
            if half > 0:
                nc.scalar.dma_start(ob[:, 0:half], ob_d[:, 0:half])
                nc.gpsimd.dma_start(ob[:, half:BW], ob_d[:, half:BW])
            else:
                nc.sync.dma_start(ob[:, 0:BW], ob_d[:, 0:BW])
        nc.vector.memset(macc[:], 0.0)

        loop_cm = tc.For_i(0, loop, 1) if loop else None
        if loop_cm is not None:
            loop_cm.__enter__()
        for _rep in range(repeat):
            rd_all = rd_pool.tile([128, RTOT], bf16, tag="rda")
            for s in live:
                nc.vector.tensor_scalar(
                    out=rd_all[:, int(offs[s]):int(offs[s]) + JS[s]],
                    in0=ojn[:, 0:JS[s]],
                    scalar1=bc[:, s:s + 1], scalar2=0.0,
                    op0=A.add, op1=A.max)
            for k, (a0, a1) in enumerate(achunks):
                scrap = scrap_pool.tile([128, ACHUNK], bf16, tag="sc")
                if accum_eng == "act":
                    nc.scalar.activation(
                        scrap[:, 0:a1 - a0], rd_all[:, a0:a1],
                        mybir.ActivationFunctionType.Relu,
                        accum_out=macc[:, k:k + 1])
                else:
                    nc.vector.tensor_scalar(
                        out=scrap[:, 0:a1 - a0], in0=rd_all[:, a0:a1],
                        scalar1=0.0, scalar2=0.0, op0=A.max, op1=A.add,
                        accum_out=macc[:, k:k + 1])
            for k, (b0, b1) in enumerate(bchunks):
                scrap = scrap_pool.tile([128, ACHUNK], bf16, tag="sc")
                nc.vector.tensor_scalar(
                    out=scrap[:, 0:b1 - b0], in0=ob[:, b0:b1],
                    scalar1=0.0, scalar2=0.0, op0=A.max, op1=A.add,
                    accum_out=macc[:, 32 + k:32 + k + 1])
        if loop_cm is not None:
            loop_cm.__exit__(None, None, None)
        nc.sync.dma_start(macc_d[:], macc[:])

    nc.compile()
    return nc


def _host_prep(t_all, o_all):
    t_g = np.zeros((G_PAD, D), dtype=np.float32)
    o_g = np.zeros((G_PAD, D), dtype=np.float32)
    t_g[:G_REAL] = t_all.reshape(G_REAL, D)
    o_g[:G_REAL] = o_all.reshape(G_REAL, D)
    idx = np.argsort(t_g, axis=1)
    t_g = np.take_along_axis(t_g, idx, axis=1)
    o_g = np.take_along_axis(o_g, idx, axis=1)

    tbf = t_g.astype(BF16).astype(np.float32)
    tbv = (t_g - np.float32(0.1)).astype(BF16).astype(np.float32)
    c = np.empty((G_PAD, D), dtype=np.int64)
    for g in range(G_REAL):
        c[g] = np.searchsorted(tbf[g], tbv[g], side="left")
    c[G_REAL:] = 0

    # c rows per (core-partition, slice): i = 2s + par, p = 2*g_local + par
    # slice extents: global across cores (single SPMD program)
    cr = c[:G_REAL].reshape(G_REAL, NS, 2)       # [g, s, par]
    cmax = cr.max(axis=(0, 2))
    cmin = cr.min(axis=(0, 2))
    JS, LS = [], []
    for s in range(NS):
        if int(cmax[s]) == 0:
            JS.append(0)
            LS.append(0)
            continue
        j = min(D, -(-(int(cmax[s]) + 1) // 8) * 8)   # pad to 8 for aligned
        l = max(0, min(int(cmin[s]) - MARGIN, j))     # offsets in rd_all
        JS.append(j)
        LS.append(l)
    return t_g, o_g, c, JS, LS


def _prep_core_inputs(t_g, o_g, c, JS, LS, core):
    """Build per-core inputs. t_g/o_g/c: full [G_PAD, D] sorted arrays."""
    g0 = core * GPC
    o_c = o_g[g0:g0 + GPC]                        # [GPC, D]
    is_last = core == N_CORES - 1

    ojn = np.repeat((-o_c).astype(BF16), 2, axis=0)     # [128, D], p = 2g+par

    # BC[p, s] = 1 + o[g, 2s+par]; pad group (last core, g_local 63) -> -1000
    ov = (np.float32(1.0) + o_c)                  # [GPC, D]
    if is_last:
        ov[GPC - 1, :] = -1000.0
    bcv = ov.reshape(GPC, NS, 2)                  # [g, s, par]
    bc = np.ascontiguousarray(
        bcv.transpose(0, 2, 1).reshape(128, NS)).astype(np.float32)

    # band OB: per live slice s, cols [L_s, J_s): value b - o_j where
    # inactive (j >= c[g, i]), else -100 (relu -> 0). Packed contiguously.
    c_c = c[g0:g0 + GPC]                          # [GPC, D]
    obs = []
    for s in range(NS):
        if JS[s] == 0:
            continue
        L, J = LS[s], JS[s]
        W = J - L
        j_idx = np.arange(L, J)
        # [GPC, 2, W]
        b_slab = bcv[:, s, :][:, :, None]         # [g, par, 1]
        o_slab = o_c[:, L:J][:, None, :]          # [g, 1, W]
        vals = (b_slab - o_slab).astype(np.float32)
        cc = c_c[:, 2 * s:2 * s + 2][:, :, None]  # [g, par, 1]
        inactive = j_idx[None, None, :] >= cc
        vals = np.where(inactive, vals, np.float32(-100.0))
        obs.append(vals.reshape(128, W))
    ob = (np.concatenate(obs, axis=1) if obs
          else np.zeros((128, 1), np.float32)).astype(BF16)
    return {"ojn": ojn, "bc": bc, "ob": np.ascontiguousarray(ob)}


def combine(res):
    total = np.float64(0.0)
    for cc in range(N_CORES):
        m = res.results[cc]["m_acc"].astype(np.float64)
        total += m[:, :32].sum() - m[:, 32:].sum()
    return 2.0 * total / float(N_PAIRS)


def kernel(input, gdt_ts):
    o_all = np.asarray(input).reshape(-1)[: G_REAL * D].astype(np.float32, copy=False)
    t_all = np.asarray(gdt_ts).reshape(-1)[: G_REAL * D].astype(np.float32, copy=False)

    t_g, o_g, c, JS, LS = _host_prep(t_all, o_all)
    in_maps = [_prep_core_inputs(t_g, o_g, c, JS, LS, cc) for cc in range(N_CORES)]
    BW = in_maps[0]["ob"].shape[1]
    for m in in_maps:
        assert m["ob"].shape[1] == BW

    key = (tuple(JS), tuple(LS), BW)
    if _CACHED.get("key") != key:
        _CACHED.update(key=key, nc=_build_program(JS, LS, BW))
    res = run_bass_kernel_spmd(_CACHED["nc"], in_maps, list(range(N_CORES)))
    return np.array([combine(res)], dtype=np.float32)


# revision 19
# speedup vs baseline: 1344.4008x; 2.5443x over previous
"""BatchRankingLoss on TRN2 — PE hinge matmuls + chunked relu-accum +
host-masked band correction.

loss = 2/N * sum_{t_i - t_j > 0.1} relu(1 + o_i - o_j)   (pair symmetry;
groups host-sorted by t so active pairs have j < i, bounded by
c[g,i] = #{j: bf16(t_j) < bf16(t_i - 0.1)}).

Layout: 64 groups/core, partition p = (g, par); slice s covers i in
{4s..4s+3} (par + 2 column-interleaved slots e); free col = 2j + e.

Device:
  PE:  D = (1 + o_i) - o_j over j < J_s via one K=68 matmul per slice
       (64 indicator rows + 2 t rows [unused] + 2 o rows), PSUM chunks.
  ACT/DVE: per chunk relu(D) with accum_out -> S1 (includes inactive
       cols c <= j < J_s).
  DVE: correction relu+accum over OB -> S2, where host packs the
       inactive band cells [L_s, J_s) as bf16(b - o_j), active -> -100.
Host: loss = 2 * (S1 - S2) / N.
"""

import numpy as np
from contextlib import ExitStack

import concourse.bacc as bacc
import concourse.mybir as mybir
import concourse.tile as tile
from concourse.bass_utils import run_bass_kernel_spmd
import ml_dtypes

BF16 = ml_dtypes.bfloat16

N_CORES = 8
D = 256
G_REAL = 511
G_PAD = 512
GPC = 64
NS = 64                   # slices per core, 4 i-values each
KDIM = 68
N_PAIRS = G_REAL * D * (D - 1)
MARGIN = 2
PSUM_F32 = 512
ACHUNK = 2048
_CACHED = {}


def _pack_chunks(exts, cap=PSUM_F32):
    chunks, cur, tot = [], [], 0
    for s, e in enumerate(exts):
        if e == 0:
            continue
        if tot + e > cap and cur:
            chunks.append(cur)
            cur, tot = [], 0
        cur.append(s)
        tot += e
    if cur:
        chunks.append(cur)
    return chunks


def _build_program(JS, LS, BW, repeat=1, loop=0):
    nc = bacc.Bacc("TRN2", target_bir_lowering=False, debug=False,
                   num_devices=N_CORES)
    f32 = mybir.dt.float32
    bf16 = mybir.dt.bfloat16
    A = mybir.AluOpType

    ES = [2 * j for j in JS]
    chunks = _pack_chunks(ES)
    NCH = len(chunks)

    # static engine split for the relu chunks: ACT ~800ns per 512-col
    # chunk, DVE ~650ns; DVE also carries the band (~2us)
    relu_eng = {}
    la, lv = 0.0, 2000.0 + BW * 0.45
    for ci, ch in enumerate(chunks):
        cols = sum(ES[s] for s in ch)
        ca = cols * 0.833 + 370.0
        cv = cols * 1.042 + 120.0
        if la + ca <= lv + cv:
            relu_eng[ci] = "act"
            la += ca
        else:
            relu_eng[ci] = "dve"
            lv += cv

    bchunks = []
    b0 = 0
    while b0 < BW:
        bchunks.append((b0, min(BW, b0 + ACHUNK)))
        b0 += ACHUNK
    assert NCH <= 48 and len(bchunks) <= 16

    ind_d = nc.dram_tensor("ind", [GPC, 128], bf16, kind="ExternalInput")
    wvo_d = nc.dram_tensor("wv_o", [2, NS * 128], bf16, kind="ExternalInput")
    ro_d = nc.dram_tensor("ro", [KDIM, 512], bf16, kind="ExternalInput")
    ob_d = nc.dram_tensor("ob", [128, max(BW, 1)], bf16, kind="ExternalInput")
    macc_d = nc.dram_tensor("m_acc", [128, 64], f32, kind="ExternalOutput")

    with ExitStack() as ctx:
        tc = ctx.enter_context(tile.TileContext(nc, num_cores=N_CORES))
        consts = ctx.enter_context(tc.tile_pool(name="consts", bufs=1))
        psd_pool = ctx.enter_context(tc.tile_pool(name="psd", bufs=4, space="PSUM"))
        rd_pool = ctx.enter_context(tc.tile_pool(name="rd", bufs=4))
        scrap_pool = ctx.enter_context(tc.tile_pool(name="scrap", bufs=2))

        w = consts.tile([KDIM, NS * 128], bf16)
        g_small = consts.tile([GPC, 128], bf16)
        ro = consts.tile([KDIM, 512], bf16)
        ob = consts.tile([128, max(BW, 1)], bf16)
        macc = consts.tile([128, 64], f32)

        nc.sync.dma_start(g_small[:], ind_d[:])
        nc.sync.dma_start(ro[:], ro_d[:])
        nc.sync.dma_start(w[GPC + 2:KDIM, :], wvo_d[:])
        nc.vector.memset(w[GPC:GPC + 2, :], 0.0)
        if BW > 0:
            half = (BW // 2) & ~3
            nc.scalar.dma_start(ob[:, 0:half], ob_d[:, 0:half])
            nc.gpsimd.dma_start(ob[:, half:BW], ob_d[:, half:BW])
        nc.vector.memset(macc[:], 0.0)

        # replicate indicator block into all slice stationaries
        PIECES = [(nc.vector, 0, 16), (nc.gpsimd, 16, 40), (nc.scalar, 40, 64)]
        for engine, s0, s1 in PIECES:
            n = s1 - s0
            src = g_small[:, None, :].broadcast_to((GPC, n, 128))
            dst = w[0:GPC, s0 * 128:s1 * 128]
            if engine is nc.scalar:
                engine.copy(dst.rearrange("g (r p) -> g r p", p=128), src)
            else:
                engine.tensor_copy(dst.rearrange("g (r p) -> g r p", p=128), src)

        loop_cm = tc.For_i(0, loop, 1) if loop else None
        if loop_cm is not None:
            loop_cm.__enter__()
        for _rep in range(repeat):
            for ci, ch in enumerate(chunks):
                ext = sum(ES[s] for s in ch)
                psd = psd_pool.tile([128, PSUM_F32], f32, tag="d")
                off = 0
                for s in ch:
                    nc.tensor.matmul(
                        psd[:, off:off + ES[s]],
                        lhsT=w[0:KDIM, s * 128:(s + 1) * 128],
                        rhs=ro[0:KDIM, 0:ES[s]],
                        start=True, stop=True,
                    )
                    off += ES[s]
                rd = rd_pool.tile([128, PSUM_F32], bf16, tag="rd")
                acc = macc[:, ci:ci + 1]
                if relu_eng[ci] == "act":
                    nc.scalar.activation(
                        rd[:, 0:ext], psd[:, 0:ext],
                        mybir.ActivationFunctionType.Relu, accum_out=acc)
                else:
                    nc.vector.tensor_scalar(
                        out=rd[:, 0:ext], in0=psd[:, 0:ext],
                        scalar1=0.0, scalar2=0.0, op0=A.max, op1=A.add,
                        accum_out=acc)
            for k, (b0, b1) in enumerate(bchunks):
                scrap = scrap_pool.tile([128, ACHUNK], bf16, tag="sc")
                nc.vector.tensor_scalar(
                    out=scrap[:, 0:b1 - b0], in0=ob[:, b0:b1],
                    scalar1=0.0, scalar2=0.0, op0=A.max, op1=A.add,
                    accum_out=macc[:, 48 + k:48 + k + 1])
        if loop_cm is not None:
            loop_cm.__exit__(None, None, None)
        nc.sync.dma_start(macc_d[:], macc[:])

    nc.compile()
    return nc


def _host_prep(t_all, o_all):
    t_g = np.zeros((G_PAD, D), dtype=np.float32)
    o_g = np.zeros((G_PAD, D), dtype=np.float32)
    t_g[:G_REAL] = t_all.reshape(G_REAL, D)
    o_g[:G_REAL] = o_all.reshape(G_REAL, D)
    idx = np.argsort(t_g, axis=1)
    t_g = np.take_along_axis(t_g, idx, axis=1)
    o_g = np.take_along_axis(o_g, idx, axis=1)

    tbf = t_g.astype(BF16).astype(np.float32)
    tbv = (t_g - np.float32(0.1)).astype(BF16).astype(np.float32)
    c = np.empty((G_PAD, D), dtype=np.int64)
    for g in range(G_REAL):
        c[g] = np.searchsorted(tbf[g], tbv[g], side="left")
    c[G_REAL:] = 0

    cr = c[:G_REAL].reshape(G_REAL, NS, 4)       # [g, s, 4i]
    cmax = cr.max(axis=(0, 2))
    cmin = cr.min(axis=(0, 2))
    JS, LS = [], []
    for s in range(NS):
        if int(cmax[s]) == 0:
            JS.append(0)
            LS.append(0)
            continue
        j = min(D, int(cmax[s]) + 1)
        l = max(0, min(int(cmin[s]) - MARGIN, j))
        JS.append(j)
        LS.append(l)
    return t_g, o_g, c, JS, LS


def _prep_core_inputs(t_g, o_g, c, JS, LS, core):
    g0 = core * GPC
    o_c = o_g[g0:g0 + GPC]
    is_last = core == N_CORES - 1

    ov = (np.float32(1.0) + o_c)
    if is_last:
        ov[GPC - 1, :] = -1000.0
    # value rows (o): [e, s*128 + (2g+par)] = ov[g, 4s+2e+par]
    a = ov.astype(BF16).astype(np.float32).reshape(GPC, NS, 2, 2)
    wv_o = np.ascontiguousarray(
        a.transpose(2, 1, 0, 3).reshape(2, NS * 128)).astype(BF16)

    ind = (np.arange(128)[None, :] // 2 ==
           np.arange(GPC)[:, None]).astype(BF16)

    ro = np.zeros((KDIM, 512), dtype=BF16)
    mo = (-o_c).astype(BF16)
    ro[:GPC, 0::2] = mo
    ro[:GPC, 1::2] = mo
    ro[GPC + 2, 0::2] = BF16(1.0)
    ro[GPC + 3, 1::2] = BF16(1.0)

    # band OB: per live slice, cols (e, j) for j in [L, J): inactive ->
    # bf16(b - o_j), active -> -100. Packed [128, sum 2W].
    c_c = c[g0:g0 + GPC]
    bcv = a  # [g, s, e, par] = bf16-rounded 1 + o_i (or -1000 pad)
    obs = []
    for s in range(NS):
        if JS[s] == 0:
            continue
        L, J = LS[s], JS[s]
        W = J - L
        j_idx = np.arange(L, J)
        b_slab = bcv[:, s, :, :].reshape(GPC, 2, 2, 1)       # [g, e, par, 1]
        o_slab = o_c[:, None, L:J].astype(BF16).astype(np.float32)
        o_slab = o_slab.reshape(GPC, 1, 1, W)
        vals = (b_slab - o_slab).astype(np.float32)          # [g, e, par, W]
        ii = (4 * s + 2 * np.arange(2)[None, :, None] +
              np.arange(2)[None, None, :])                   # [1, e, par]
        cc = np.take_along_axis(
            c_c[:, :], np.broadcast_to(ii, (GPC, 2, 2)).reshape(GPC, 4),
            axis=1).reshape(GPC, 2, 2, 1)
        inactive = j_idx[None, None, None, :] >= cc
        vals = np.where(inactive, vals, np.float32(-100.0))
        # -> [p = 2g+par, e*W + w]
        vals = vals.transpose(0, 2, 1, 3).reshape(128, 2 * W)
        obs.append(vals)
    ob = (np.concatenate(obs, axis=1) if obs
          else np.zeros((128, 1), np.float32)).astype(BF16)
    return {"ind": ind, "wv_o": wv_o, "ro": ro,
            "ob": np.ascontiguousarray(ob)}


def combine(res):
    total = np.float64(0.0)
    for cc in range(N_CORES):
        m = res.results[cc]["m_acc"].astype(np.float64)
        total += m[:, :48].sum() - m[:, 48:].sum()
    return 2.0 * total / float(N_PAIRS)


def kernel(input, gdt_ts):
    o_all = np.asarray(input).reshape(-1)[: G_REAL * D].astype(np.float32, copy=False)
    t_all = np.asarray(gdt_ts).reshape(-1)[: G_REAL * D].astype(np.float32, copy=False)

    t_g, o_g, c, JS, LS = _host_prep(t_all, o_all)
    in_maps = [_prep_core_inputs(t_g, o_g, c, JS, LS, cc) for cc in range(N_CORES)]
    BW = in_maps[0]["ob"].shape[1]
    for m in in_maps:
        assert m["ob"].shape[1] == BW

    key = (tuple(JS), tuple(LS), BW)
    if _CACHED.get("key") != key:
        _CACHED.update(key=key, nc=_build_program(JS, LS, BW))
    res = run_bass_kernel_spmd(_CACHED["nc"], in_maps, list(range(N_CORES)))
    return np.array([combine(res)], dtype=np.float32)


# revision 20
# speedup vs baseline: 1496.0370x; 1.1128x over previous
"""BatchRankingLoss on TRN2 — PE hinge matmuls + chunked relu-accum +
host-masked band correction.

loss = 2/N * sum_{t_i - t_j > 0.1} relu(1 + o_i - o_j)   (pair symmetry;
groups host-sorted by t so active pairs have j < i, bounded by
c[g,i] = #{j: bf16(t_j) < bf16(t_i - 0.1)}).

Layout: 64 groups/core, partition p = (g, par); slice s covers i in
{4s..4s+3} (par + 2 column-interleaved slots e); free col = 2j + e.

Device:
  PE:  D = (1 + o_i) - o_j over j < J_s via one K=68 matmul per slice
       (64 indicator rows + 2 t rows [unused] + 2 o rows), PSUM chunks.
  ACT/DVE: per chunk relu(D) with accum_out -> S1 (includes inactive
       cols c <= j < J_s).
  DVE: correction relu+accum over OB -> S2, where host packs the
       inactive band cells [L_s, J_s) as bf16(b - o_j), active -> -100.
Host: loss = 2 * (S1 - S2) / N.
"""

import numpy as np
from contextlib import ExitStack

import concourse.bacc as bacc
import concourse.mybir as mybir
import concourse.tile as tile
from concourse.bass_utils import run_bass_kernel_spmd
import ml_dtypes

BF16 = ml_dtypes.bfloat16

N_CORES = 8
D = 256
G_REAL = 511
G_PAD = 512
GPC = 64
NS = 64                   # slices per core, 4 i-values each
KDIM = 66
N_PAIRS = G_REAL * D * (D - 1)
MARGIN = 2
PSUM_F32 = 512
ACHUNK = 2048
_CACHED = {}


def _pack_chunks(exts, cap=PSUM_F32):
    chunks, cur, tot = [], [], 0
    for s, e in enumerate(exts):
        if e == 0:
            continue
        if tot + e > cap and cur:
            chunks.append(cur)
            cur, tot = [], 0
        cur.append(s)
        tot += e
    if cur:
        chunks.append(cur)
    return chunks


def _build_program(JS, LS, BW, repeat=1, loop=0):
    nc = bacc.Bacc("TRN2", target_bir_lowering=False, debug=False,
                   num_devices=N_CORES)
    f32 = mybir.dt.float32
    bf16 = mybir.dt.bfloat16
    A = mybir.AluOpType

    ES = [2 * j for j in JS]
    chunks = _pack_chunks(ES)
    NCH = len(chunks)

    # static engine split for the relu chunks: ACT ~800ns per 512-col
    # chunk, DVE ~650ns; DVE also carries the band (~2us)
    relu_eng = {}
    la, lv = 0.0, 2000.0 + BW * 0.45
    for ci, ch in enumerate(chunks):
        cols = sum(ES[s] for s in ch)
        ca = cols * 0.833 + 370.0
        cv = cols * 1.042 + 120.0
        if la + ca <= lv + cv:
            relu_eng[ci] = "act"
            la += ca
        else:
            relu_eng[ci] = "dve"
            lv += cv

    bchunks = []
    b0 = 0
    while b0 < BW:
        bchunks.append((b0, min(BW, b0 + ACHUNK)))
        b0 += ACHUNK
    assert NCH <= 48 and len(bchunks) <= 16

    ind_d = nc.dram_tensor("ind", [GPC, NS * 128], bf16, kind="ExternalInput")
    wvo_d = nc.dram_tensor("wv_o", [2, NS * 128], bf16, kind="ExternalInput")
    ro_d = nc.dram_tensor("ro", [KDIM, 512], bf16, kind="ExternalInput")
    ob_d = nc.dram_tensor("ob", [128, max(BW, 1)], bf16, kind="ExternalInput")
    macc_d = nc.dram_tensor("m_acc", [128, 64], f32, kind="ExternalOutput")

    with ExitStack() as ctx:
        tc = ctx.enter_context(tile.TileContext(nc, num_cores=N_CORES))
        consts = ctx.enter_context(tc.tile_pool(name="consts", bufs=1))
        psd_pool = ctx.enter_context(tc.tile_pool(name="psd", bufs=6, space="PSUM"))
        rd_pool = ctx.enter_context(tc.tile_pool(name="rd", bufs=6))
        scrap_pool = ctx.enter_context(tc.tile_pool(name="scrap", bufs=2))

        w = consts.tile([KDIM, NS * 128], bf16)
        ro = consts.tile([KDIM, 512], bf16)
        ob = consts.tile([128, max(BW, 1)], bf16)
        macc = consts.tile([128, 64], f32)

        nc.sync.dma_start(w[0:GPC, :], ind_d[:])
        nc.sync.dma_start(ro[:], ro_d[:])
        nc.sync.dma_start(w[GPC:KDIM, :], wvo_d[:])
        if BW > 0:
            half = (BW // 2) & ~3
            nc.scalar.dma_start(ob[:, 0:half], ob_d[:, 0:half])
            nc.gpsimd.dma_start(ob[:, half:BW], ob_d[:, half:BW])
        nc.vector.memset(macc[:], 0.0)

        loop_cm = tc.For_i(0, loop, 1) if loop else None
        if loop_cm is not None:
            loop_cm.__enter__()
        for _rep in range(repeat):
            for ci, ch in enumerate(chunks):
                ext = sum(ES[s] for s in ch)
                psd = psd_pool.tile([128, PSUM_F32], f32, tag="d")
                off = 0
                for s in ch:
                    nc.tensor.matmul(
                        psd[:, off:off + ES[s]],
                        lhsT=w[0:KDIM, s * 128:(s + 1) * 128],
                        rhs=ro[0:KDIM, 0:ES[s]],
                        start=True, stop=True,
                    )
                    off += ES[s]
                rd = rd_pool.tile([128, PSUM_F32], bf16, tag="rd")
                acc = macc[:, ci:ci + 1]
                if relu_eng[ci] == "act":
                    nc.scalar.activation(
                        rd[:, 0:ext], psd[:, 0:ext],
                        mybir.ActivationFunctionType.Relu, accum_out=acc)
                else:
                    nc.vector.tensor_scalar(
                        out=rd[:, 0:ext], in0=psd[:, 0:ext],
                        scalar1=0.0, scalar2=0.0, op0=A.max, op1=A.add,
                        accum_out=acc)
            for k, (b0, b1) in enumerate(bchunks):
                scrap = scrap_pool.tile([128, ACHUNK], bf16, tag="sc")
                nc.vector.tensor_scalar(
                    out=scrap[:, 0:b1 - b0], in0=ob[:, b0:b1],
                    scalar1=0.0, scalar2=0.0, op0=A.max, op1=A.add,
                    accum_out=macc[:, 48 + k:48 + k + 1])
        if loop_cm is not None:
            loop_cm.__exit__(None, None, None)
        nc.sync.dma_start(macc_d[:], macc[:])

    nc.compile()
    return nc


def _host_prep(t_all, o_all):
    t_g = np.zeros((G_PAD, D), dtype=np.float32)
    o_g = np.zeros((G_PAD, D), dtype=np.float32)
    t_g[:G_REAL] = t_all.reshape(G_REAL, D)
    o_g[:G_REAL] = o_all.reshape(G_REAL, D)
    idx = np.argsort(t_g, axis=1)
    t_g = np.take_along_axis(t_g, idx, axis=1)
    o_g = np.take_along_axis(o_g, idx, axis=1)

    tbf = t_g.astype(BF16).astype(np.float32)
    tbv = (t_g - np.float32(0.1)).astype(BF16).astype(np.float32)
    c = np.empty((G_PAD, D), dtype=np.int64)
    for g in range(G_REAL):
        c[g] = np.searchsorted(tbf[g], tbv[g], side="left")
    c[G_REAL:] = 0

    cr = c[:G_REAL].reshape(G_REAL, NS, 4)       # [g, s, 4i]
    cmax = cr.max(axis=(0, 2))
    cmin = cr.min(axis=(0, 2))
    JS, LS = [], []
    for s in range(NS):
        if int(cmax[s]) == 0:
            JS.append(0)
            LS.append(0)
            continue
        j = min(D, int(cmax[s]) + 1)
        l = max(0, min(int(cmin[s]) - MARGIN, j))
        JS.append(j)
        LS.append(l)
    return t_g, o_g, c, JS, LS


def _prep_core_inputs(t_g, o_g, c, JS, LS, core):
    g0 = core * GPC
    o_c = o_g[g0:g0 + GPC]
    is_last = core == N_CORES - 1

    ov = (np.float32(1.0) + o_c)
    if is_last:
        ov[GPC - 1, :] = -1000.0
    # value rows (o): [e, s*128 + (2g+par)] = ov[g, 4s+2e+par]
    a = ov.astype(BF16).astype(np.float32).reshape(GPC, NS, 2, 2)
    wv_o = np.ascontiguousarray(
        a.transpose(2, 1, 0, 3).reshape(2, NS * 128)).astype(BF16)

    ind = (np.arange(128)[None, :] // 2 ==
           np.arange(GPC)[:, None]).astype(BF16)
    ind = np.ascontiguousarray(np.tile(ind, (1, NS)))

    ro = np.zeros((KDIM, 512), dtype=BF16)
    mo = (-o_c).astype(BF16)
    ro[:GPC, 0::2] = mo
    ro[:GPC, 1::2] = mo
    ro[GPC, 0::2] = BF16(1.0)
    ro[GPC + 1, 1::2] = BF16(1.0)

    # band OB: per live slice, cols (e, j) for j in [L, J): inactive ->
    # bf16(b - o_j), active -> -100. Packed [128, sum 2W].
    c_c = c[g0:g0 + GPC]
    bcv = a  # [g, s, e, par] = bf16-rounded 1 + o_i (or -1000 pad)
    obs = []
    for s in range(NS):
        if JS[s] == 0:
            continue
        L, J = LS[s], JS[s]
        W = J - L
        j_idx = np.arange(L, J)
        b_slab = bcv[:, s, :, :].reshape(GPC, 2, 2, 1)       # [g, e, par, 1]
        o_slab = o_c[:, None, L:J].astype(BF16).astype(np.float32)
        o_slab = o_slab.reshape(GPC, 1, 1, W)
        vals = (b_slab - o_slab).astype(np.float32)          # [g, e, par, W]
        ii = (4 * s + 2 * np.arange(2)[None, :, None] +
              np.arange(2)[None, None, :])                   # [1, e, par]
        cc = np.take_along_axis(
            c_c[:, :], np.broadcast_to(ii, (GPC, 2, 2)).reshape(GPC, 4),
            axis=1).reshape(GPC, 2, 2, 1)
        inactive = j_idx[None, None, None, :] >= cc
        vals = np.where(inactive, vals, np.float32(-100.0))
        # -> [p = 2g+par, e*W + w]
        vals = vals.transpose(0, 2, 1, 3).reshape(128, 2 * W)
        obs.append(vals)
    ob = (np.concatenate(obs, axis=1) if obs
          else np.zeros((128, 1), np.float32)).astype(BF16)
    return {"ind": ind, "wv_o": wv_o, "ro": ro,
            "ob": np.ascontiguousarray(ob)}


def combine(res):
    total = np.float64(0.0)
    for cc in range(N_CORES):
        m = res.results[cc]["m_acc"].astype(np.float64)
        total += m[:, :48].sum() - m[:, 48:].sum()
    return 2.0 * total / float(N_PAIRS)


def kernel(input, gdt_ts):
    o_all = np.asarray(input).reshape(-1)[: G_REAL * D].astype(np.float32, copy=False)
    t_all = np.asarray(gdt_ts).reshape(-1)[: G_REAL * D].astype(np.float32, copy=False)

    t_g, o_g, c, JS, LS = _host_prep(t_all, o_all)
    in_maps = [_prep_core_inputs(t_g, o_g, c, JS, LS, cc) for cc in range(N_CORES)]
    BW = in_maps[0]["ob"].shape[1]
    for m in in_maps:
        assert m["ob"].shape[1] == BW

    key = (tuple(JS), tuple(LS), BW)
    if _CACHED.get("key") != key:
        _CACHED.update(key=key, nc=_build_program(JS, LS, BW))
    res = run_bass_kernel_spmd(_CACHED["nc"], in_maps, list(range(N_CORES)))
    return np.array([combine(res)], dtype=np.float32)
